# revision 7
# baseline (speedup 1.0000x reference)
"""Conv2d(128->256, 3x3, pad 1) with LoRA (rank 8) — Trainium2 Bass kernel.

Strategy:
  - Data-parallel over batch: 16 images -> 2 per core x 8 cores; weights
    replicated.
  - LoRA folds into the conv weight on the host (conv is linear in weights,
    and W_eff = W + (alpha/rank) * (B @ A) is 0.3 MFLOP vs the conv's
    4.8 GFLOP): the device kernel is a pure 3x3 conv with a bias.
  - The conv = 9 shifted matmuls accumulating in PSUM:
        out[co, pix] += W_eff[co, :, kh, kw]^T @ x_shift[ci, pix]
    with K = C_IN = 128 (partition dim), M = 128 (co block), N = 512
    (8 image rows x 64 cols) in bf16 (full PE rate, weight loads hidden).
  - x and W_eff ship as bf16 (what the PE consumes anyway): halves input
    DMA bytes and removes every pre-conv compute op from the device.
  - The PE p-state ramp needs ~3us of CONTINUOUS busy before full clock
    (and resets on idle gaps), so a short warm-up matmul burst bridges the
    initial DMA window and the conv stream is kept gap-free.
  - Tail: the last row group runs as two 256-col PSUM tiles, drained in
    halves on ACT+DVE and DMA'd on both HW queues, to shorten the
    matmul-end -> last-HBM-write path.
"""

import numpy as np

import concourse.bass as bass
import concourse.tile as tile
from concourse.tile import add_dep_helper
from concourse import bacc, mybir
from concourse.bass_utils import run_bass_kernel_spmd

N_CORES = 8
B, C_IN, H, W_DIM = 16, 128, 64, 64
C_OUT = 256
RANK = 8
SCALING = 2.0  # alpha/rank = 16/8
HP, WP = H + 2, W_DIM + 2  # zero-padded image dims
B_LOC = B // N_CORES  # images per core
NPIX = H * W_DIM  # 4096
ROWS_PER_TILE = 8  # output rows per matmul group -> N = 8*64 = 512
N_RG = H // ROWS_PER_TILE  # 8 row groups

F32 = mybir.dt.float32
BF16 = mybir.dt.bfloat16
IDENT = mybir.ActivationFunctionType.Identity


def _build_nc():
    nc = bacc.Bacc(
        "TRN2",
        target_bir_lowering=False,
        debug=False,
        num_devices=N_CORES,
    )

    xp = nc.dram_tensor("xp", [B_LOC, C_IN, HP * WP], BF16, kind="ExternalInput").ap()
    wt = nc.dram_tensor("wt", [C_IN, 9 * C_OUT], BF16, kind="ExternalInput").ap()
    bv = nc.dram_tensor("bv", [128, 2], F32, kind="ExternalInput").ap()
    out = nc.dram_tensor("out", [B_LOC, C_OUT, NPIX], F32, kind="ExternalOutput").ap()

    with tile.TileContext(nc) as tc:
        with (
            tc.tile_pool(name="persist", bufs=1) as persist,
            tc.tile_pool(name="outp", bufs=6) as outp,
            tc.tile_pool(name="psum", bufs=7, space="PSUM") as psum,
        ):
            # --- persistent SBUF tiles -------------------------------------
            x_sb = [
                persist.tile([C_IN, HP * WP], BF16, name=f"x_sb{i}")
                for i in range(B_LOC)
            ]
            weff = persist.tile([C_IN, 9 * C_OUT], BF16, name="weff")
            b_sb = persist.tile([128, 2], F32, name="b_sb")
            warm_sb = persist.tile([128, 512], BF16, name="warm_sb")

            # --- PE warm-up ------------------------------------------------
            # The PE runs at the mid p-state until it has been continuously
            # busy ~3us (and the ramp resets on idle gaps). These dummy
            # matmuls start the ramp during the input prefetch and keep the
            # PE busy until the first weff/x chunks land.
            nc.gpsimd.dma_start(b_sb[:], bv)
            nc.gpsimd.memset(warm_sb[:], 0.0)
            warm_ps = psum.tile([128, 512], F32, tag="warm", bufs=1, name="warm_ps")
            for _ in range(4):
                nc.tensor.matmul(
                    warm_ps[:], warm_sb[:, :128], warm_sb[:], start=True, stop=True
                )

            # --- input DMAs ------------------------------------------------
            # Startup is HBM-bandwidth-critical: only bytes the first conv
            # groups need go first, interleaved across both HW queues by
            # need-time. Image 1 is needed only ~35us in, so its DMAs are
            # held back (dep on early conv tiles) to keep it from starving
            # the startup transfers.
            qs = [nc.sync, nc.scalar]
            XC = 6
            xsz = (HP * WP + XC - 1) // XC  # 726 cols = 11 image rows

            def xdma(eng, i, c):
                lo, hi = c * xsz, min((c + 1) * xsz, HP * WP)
                return eng.dma_start(x_sb[i][:, lo:hi], xp[i, :, lo:hi])

            wq = (9 * C_OUT) // 3  # 768 cols = 3 k-slices per chunk
            xdma(nc.scalar, 0, 0)
            for q in range(3):
                nc.sync.dma_start(
                    weff[:, q * wq : (q + 1) * wq], wt[:, q * wq : (q + 1) * wq]
                )
            xdma(nc.scalar, 0, 1)
            xdma(nc.scalar, 0, 3)
            xdma(nc.scalar, 0, 5)
            xdma(nc.sync, 0, 2)
            xdma(nc.sync, 0, 4)
            x1_dmas = [xdma(nc.gpsimd, 1, c) for c in range(XC)]

            # --- the conv: 9 accumulating shift-matmuls per output tile ----
            def conv_tile(x_r, cb, h0, nrows, ps):
                mm = None
                for k in range(9):
                    dh, dw = k // 3 - 1, k % 3 - 1
                    rhs = x_r[
                        :,
                        h0 + 1 + dh : h0 + 1 + dh + nrows,
                        1 + dw : 65 + dw,
                    ]
                    lhsT = weff[:, k * 256 + cb * 128 : k * 256 + cb * 128 + 128]
                    mm = nc.tensor.matmul(
                        ps[:], lhsT, rhs, start=(k == 0), stop=(k == 8)
                    )
                return mm

            for img in range(B_LOC):
                x_r = x_sb[img][:].rearrange("p (h w) -> p h w", w=WP)
                for cb in range(2):
                    for rg in range(N_RG):
                        ti = (img * 2 + cb) * N_RG + rg
                        h0 = rg * ROWS_PER_TILE
                        dst = out[
                            img, cb * 128 : (cb + 1) * 128, rg * 512 : (rg + 1) * 512
                        ]
                        if ti < 31:
                            ps = psum.tile([128, 512], F32, tag="ps", name=f"ps{ti}")
                            mm = conv_tile(x_r, cb, h0, ROWS_PER_TILE, ps)
                            if ti < len(x1_dmas):
                                # release image-1's DMA only once the startup
                                # transfers are out of the way
                                add_dep_helper(
                                    x1_dmas[ti].ins,
                                    mm.ins,
                                    reason="defer x1 traffic past startup",
                                )
                            o = outp.tile([128, 512], F32, tag="o", name=f"o{ti}")
                            if ti < 29:
                                # Alternate the PSUM->SBUF bias-add between
                                # ACT and DVE so neither engine limits the
                                # PSUM drain.
                                if ti % 2 == 0:
                                    nc.scalar.activation(
                                        o[:], ps[:], IDENT, bias=b_sb[:, cb : cb + 1]
                                    )
                                else:
                                    nc.vector.tensor_scalar_add(
                                        o[:], ps[:], b_sb[:, cb : cb + 1]
                                    )
                                qs[ti % 2].dma_start(dst, o[:])
                            else:
                                nc.scalar.activation(
                                    o[:, :256],
                                    ps[:, :256],
                                    IDENT,
                                    bias=b_sb[:, cb : cb + 1],
                                )
                                nc.vector.tensor_scalar_add(
                                    o[:, 256:], ps[:, 256:], b_sb[:, cb : cb + 1]
                                )
                                qs[0].dma_start(dst[:, :256], o[:, :256])
                                qs[1].dma_start(dst[:, 256:], o[:, 256:])
                        else:
                            # Final row group: two 256-col sub-tiles so the
                            # very last matmul->drain->DMA chain is short.
                            for hf in range(2):
                                ps = psum.tile(
                                    [128, 256], F32, tag="ps", name=f"ps{ti}_{hf}"
                                )
                                conv_tile(x_r, cb, h0 + hf * 4, 4, ps)
                                o = outp.tile(
                                    [128, 256], F32, tag="o", name=f"o{ti}_{hf}"
                                )
                                nc.scalar.activation(
                                    o[:, :128],
                                    ps[:, :128],
                                    IDENT,
                                    bias=b_sb[:, cb : cb + 1],
                                )
                                nc.vector.tensor_scalar_add(
                                    o[:, 128:], ps[:, 128:], b_sb[:, cb : cb + 1]
                                )
                                half = dst[:, hf * 256 : (hf + 1) * 256]
                                qs[0].dma_start(half[:, :128], o[:, :128])
                                qs[1].dma_start(half[:, 128:], o[:, 128:])

    nc.compile()
    return nc


_NC_CACHE = None


def _get_nc():
    global _NC_CACHE
    if _NC_CACHE is None:
        _NC_CACHE = _build_nc()
    return _NC_CACHE


def _host_prep(x, W, b, lora_A, lora_B):
    """Host prep: fold LoRA into the conv weight, pad, transpose, bf16."""
    bf16 = mybir.dt.np(BF16)
    x = np.asarray(x, dtype=np.float32)
    xp_all = np.zeros((B, C_IN, HP, WP), dtype=bf16)
    xp_all[:, :, 1 : H + 1, 1 : W_DIM + 1] = x.astype(bf16)
    xp_all = xp_all.reshape(B, C_IN, HP * WP)

    # W_eff = W + (alpha/rank) * (B @ A), then [co, ci, k] -> [ci, k, co]
    weff = (
        np.asarray(W, dtype=np.float32).reshape(C_OUT, C_IN * 9)
        + SCALING
        * (np.asarray(lora_B, np.float32) @ np.asarray(lora_A, np.float32))
    )
    wt = np.ascontiguousarray(
        weff.reshape(C_OUT, C_IN, 9).transpose(1, 2, 0)
    ).reshape(C_IN, 9 * C_OUT).astype(bf16)
    # [256] -> [128, 2]: bv[p, cb] = b[cb*128 + p]
    bv = np.ascontiguousarray(np.asarray(b, dtype=np.float32).reshape(2, 128).T)
    return xp_all, wt, bv


def run(x, W, b, lora_A, lora_B, trace=False):
    """Run the kernel on 8 cores; returns (full_output, BassKernelResults)."""
    xp_all, wt, bv = _host_prep(x, W, b, lora_A, lora_B)
    nc = _get_nc()
    in_maps = []
    for c in range(N_CORES):
        in_maps.append(
            {
                "xp": np.ascontiguousarray(xp_all[c * B_LOC : (c + 1) * B_LOC]),
                "wt": wt,
                "bv": bv,
            }
        )
    res = run_bass_kernel_spmd(
        nc, in_maps, core_ids=list(range(N_CORES)), trace=trace
    )
    out = np.concatenate([r["out"] for r in res.results], axis=0)
    return out.reshape(B, C_OUT, H, W_DIM), res


def kernel(x, W, b, lora_A, lora_B):
    out, _ = run(x, W, b, lora_A, lora_B, trace=False)
    return out


# revision 14
# speedup vs baseline: 1.1504x; 1.1504x over previous
"""Conv2d(128->256, 3x3, pad 1) with LoRA (rank 8) — Trainium2 Bass kernel.

Strategy:
  - Data-parallel over batch: 16 images -> 2 per core x 8 cores; weights
    replicated.
  - LoRA folds into the conv weight on the host (conv is linear in weights,
    and W_eff = W + (alpha/rank) * (B @ A) is 0.3 MFLOP vs the conv's
    4.8 GFLOP): the device kernel is a pure 3x3 conv with a bias.
  - The conv = 9 shifted matmuls accumulating in PSUM:
        out[co, pix] += W_eff[co, :, kh, kw]^T @ x_shift[ci, pix]
    with K = C_IN = 128 (partition dim), M = 128 (co block), N = 512
    (8 image rows x 64 cols) in bf16 (full PE rate, weight loads hidden).
  - x and W_eff ship as bf16 (what the PE consumes anyway): halves input
    DMA bytes and removes every pre-conv compute op from the device.
  - The PE p-state ramp needs ~3us of CONTINUOUS busy before full clock
    (and resets on idle gaps), so a short warm-up matmul burst bridges the
    initial DMA window and the conv stream is kept gap-free.
  - Tail: the last row group runs as two 256-col PSUM tiles, drained in
    halves on ACT+DVE and DMA'd on both HW queues, to shorten the
    matmul-end -> last-HBM-write path.
"""

import numpy as np

import concourse.bass as bass
import concourse.tile as tile
from concourse.tile import add_dep_helper
from concourse import bacc, mybir
from concourse.bass_utils import run_bass_kernel_spmd

N_CORES = 8
B, C_IN, H, W_DIM = 16, 128, 64, 64
C_OUT = 256
RANK = 8
SCALING = 2.0  # alpha/rank = 16/8
HP, WP = H + 2, W_DIM + 2  # zero-padded image dims
B_LOC = B // N_CORES  # images per core
NPIX = H * W_DIM  # 4096
ROWS_PER_TILE = 8  # output rows per matmul group -> N = 8*64 = 512
N_RG = H // ROWS_PER_TILE  # 8 row groups

F32 = mybir.dt.float32
BF16 = mybir.dt.bfloat16
IDENT = mybir.ActivationFunctionType.Identity


def _build_nc():
    nc = bacc.Bacc(
        "TRN2",
        target_bir_lowering=False,
        debug=False,
        num_devices=N_CORES,
    )

    # wt = [bias (2 cols) | weff (9*256 cols)] in bf16: the bias rides the
    # first weff chunk instead of needing its own (slow) tiny DMA.
    WCOLS = 2 + 9 * C_OUT
    xp = nc.dram_tensor("xp", [B_LOC, C_IN, HP * WP], BF16, kind="ExternalInput").ap()
    wt = nc.dram_tensor("wt", [C_IN, WCOLS], BF16, kind="ExternalInput").ap()
    out = nc.dram_tensor("out", [B_LOC, C_OUT, NPIX], F32, kind="ExternalOutput").ap()

    with tile.TileContext(nc) as tc:
        with (
            tc.tile_pool(name="persist", bufs=1) as persist,
            tc.tile_pool(name="outp", bufs=6) as outp,
            tc.tile_pool(name="psum", bufs=7, space="PSUM") as psum,
        ):
            # --- persistent SBUF tiles -------------------------------------
            x_sb = [
                persist.tile([C_IN, HP * WP], BF16, name=f"x_sb{i}")
                for i in range(B_LOC)
            ]
            weff = persist.tile([C_IN, WCOLS], BF16, name="weff")
            b32 = persist.tile([128, 2], F32, name="b32")
            warm_sb = persist.tile([128, 256], BF16, name="warm_sb")

            # --- PE warm-up ------------------------------------------------
            # The PE runs at the mid p-state until it has been continuously
            # busy ~3us (and the ramp resets on idle gaps). These dummy
            # matmuls start the ramp during the input prefetch and keep the
            # PE busy until the first weff/x chunks land.
            nc.gpsimd.memset(warm_sb[:], 0.0)
            warm_ps = psum.tile([128, 256], F32, tag="warm", bufs=1, name="warm_ps")
            for _ in range(6):
                nc.tensor.matmul(
                    warm_ps[:], warm_sb[:, :128], warm_sb[:], start=True, stop=True
                )

            # --- input DMAs ------------------------------------------------
            # Startup is HBM-bandwidth-critical: only bytes the first conv
            # groups need go first, interleaved across both HW queues by
            # need-time. Image 1 is needed only ~35us in, so its DMAs are
            # held back (dep on early conv tiles) to keep it from starving
            # the startup transfers.
            qs = [nc.sync, nc.scalar]
            XC = 6
            xsz = (HP * WP + XC - 1) // XC  # 726 cols = 11 image rows

            def xdma(eng, i, c):
                lo, hi = c * xsz, min((c + 1) * xsz, HP * WP)
                return eng.dma_start(x_sb[i][:, lo:hi], xp[i, :, lo:hi])

            wsz = WCOLS // 3  # 768/770 cols: chunk 0 = bias + k 0..2
            wbounds = [0, wsz + 2, 2 * wsz + 2, WCOLS]
            xdma(nc.scalar, 0, 0)
            for q in range(3):
                lo, hi = wbounds[q], wbounds[q + 1]
                nc.sync.dma_start(weff[:, lo:hi], wt[:, lo:hi])
            xdma(nc.scalar, 0, 1)
            xdma(nc.scalar, 0, 3)
            xdma(nc.scalar, 0, 5)
            xdma(nc.sync, 0, 2)
            xdma(nc.sync, 0, 4)
            x1_dmas = [xdma(nc.gpsimd, 1, c) for c in range(XC)]
            # bias shipped as the first 2 bf16 cols of wt; widen once for the
            # f32-only DVE/ACT bias ports
            nc.vector.tensor_copy(b32[:], weff[:, 0:2])

            # --- the conv: 9 accumulating shift-matmuls per output tile ----
            def conv_tile(x_r, cb, h0, nrows, ps):
                mm = None
                for k in range(9):
                    dh, dw = k // 3 - 1, k % 3 - 1
                    rhs = x_r[
                        :,
                        h0 + 1 + dh : h0 + 1 + dh + nrows,
                        1 + dw : 65 + dw,
                    ]
                    co0 = 2 + k * 256 + cb * 128
                    lhsT = weff[:, co0 : co0 + 128]
                    mm = nc.tensor.matmul(
                        ps[:], lhsT, rhs, start=(k == 0), stop=(k == 8)
                    )
                return mm

            for img in range(B_LOC):
                x_r = x_sb[img][:].rearrange("p (h w) -> p h w", w=WP)
                for cb in range(2):
                    for rg in range(N_RG):
                        ti = (img * 2 + cb) * N_RG + rg
                        h0 = rg * ROWS_PER_TILE
                        dst = out[
                            img, cb * 128 : (cb + 1) * 128, rg * 512 : (rg + 1) * 512
                        ]
                        if ti < 31:
                            ps = psum.tile([128, 512], F32, tag="ps", name=f"ps{ti}")
                            mm = conv_tile(x_r, cb, h0, ROWS_PER_TILE, ps)
                            if ti < len(x1_dmas):
                                # release image-1's DMA only once the startup
                                # transfers are out of the way
                                add_dep_helper(
                                    x1_dmas[ti].ins,
                                    mm.ins,
                                    reason="defer x1 traffic past startup",
                                )
                            o = outp.tile([128, 512], F32, tag="o", name=f"o{ti}")
                            if ti < 29:
                                # Alternate the PSUM->SBUF bias-add between
                                # ACT and DVE so neither engine limits the
                                # PSUM drain.
                                if ti % 2 == 0:
                                    nc.scalar.activation(
                                        o[:], ps[:], IDENT, bias=b32[:, cb : cb + 1]
                                    )
                                else:
                                    nc.vector.tensor_scalar_add(
                                        o[:], ps[:], b32[:, cb : cb + 1]
                                    )
                                qs[ti % 2].dma_start(dst, o[:])
                            else:
                                nc.scalar.activation(
                                    o[:, :256],
                                    ps[:, :256],
                                    IDENT,
                                    bias=b32[:, cb : cb + 1],
                                )
                                nc.vector.tensor_scalar_add(
                                    o[:, 256:], ps[:, 256:], b32[:, cb : cb + 1]
                                )
                                qs[0].dma_start(dst[:, :256], o[:, :256])
                                qs[1].dma_start(dst[:, 256:], o[:, 256:])
                        else:
                            # Final row group: two 256-col sub-tiles so the
                            # very last matmul->drain->DMA chain is short.
                            for hf in range(2):
                                ps = psum.tile(
                                    [128, 256], F32, tag="ps", name=f"ps{ti}_{hf}"
                                )
                                conv_tile(x_r, cb, h0 + hf * 4, 4, ps)
                                o = outp.tile(
                                    [128, 256], F32, tag="o", name=f"o{ti}_{hf}"
                                )
                                nc.scalar.activation(
                                    o[:, :128],
                                    ps[:, :128],
                                    IDENT,
                                    bias=b32[:, cb : cb + 1],
                                )
                                nc.vector.tensor_scalar_add(
                                    o[:, 128:], ps[:, 128:], b32[:, cb : cb + 1]
                                )
                                half = dst[:, hf * 256 : (hf + 1) * 256]
                                qs[0].dma_start(half[:, :128], o[:, :128])
                                qs[1].dma_start(half[:, 128:], o[:, 128:])

    nc.compile()
    return nc


_NC_CACHE = None


def _get_nc():
    global _NC_CACHE
    if _NC_CACHE is None:
        _NC_CACHE = _build_nc()
    return _NC_CACHE


def _host_prep(x, W, b, lora_A, lora_B):
    """Host prep: fold LoRA into the conv weight, pad, transpose, bf16."""
    bf16 = mybir.dt.np(BF16)
    x = np.asarray(x, dtype=np.float32)
    xp_all = np.zeros((B, C_IN, HP, WP), dtype=bf16)
    xp_all[:, :, 1 : H + 1, 1 : W_DIM + 1] = x.astype(bf16)
    xp_all = xp_all.reshape(B, C_IN, HP * WP)

    # W_eff = W + (alpha/rank) * (B @ A), then [co, ci, k] -> [ci, k, co]
    weff = (
        np.asarray(W, dtype=np.float32).reshape(C_OUT, C_IN * 9)
        + SCALING
        * (np.asarray(lora_B, np.float32) @ np.asarray(lora_A, np.float32))
    )
    wk = np.ascontiguousarray(
        weff.reshape(C_OUT, C_IN, 9).transpose(1, 2, 0)
    ).reshape(C_IN, 9 * C_OUT)
    # wt = [bias (2 cols: bv[p, cb] = b[cb*128 + p]) | weff], all bf16
    bv = np.asarray(b, dtype=np.float32).reshape(2, 128).T
    wt = np.ascontiguousarray(
        np.concatenate([bv, wk], axis=1).astype(bf16)
    )
    return xp_all, wt


def run(x, W, b, lora_A, lora_B, trace=False):
    """Run the kernel on 8 cores; returns (full_output, BassKernelResults)."""
    xp_all, wt = _host_prep(x, W, b, lora_A, lora_B)
    nc = _get_nc()
    in_maps = []
    for c in range(N_CORES):
        in_maps.append(
            {
                "xp": np.ascontiguousarray(xp_all[c * B_LOC : (c + 1) * B_LOC]),
                "wt": wt,
            }
        )
    res = run_bass_kernel_spmd(
        nc, in_maps, core_ids=list(range(N_CORES)), trace=trace
    )
    out = np.concatenate([r["out"] for r in res.results], axis=0)
    return out.reshape(B, C_OUT, H, W_DIM), res


def kernel(x, W, b, lora_A, lora_B):
    out, _ = run(x, W, b, lora_A, lora_B, trace=False)
    return out
